# revision 13
# baseline (speedup 1.0000x reference)
"""Trainium2 Bass kernel for graph-transformer message passing (TransformerConv).

Strategy (8 NeuronCores, SPMD, no collectives):
  - Host sorts edges by dst and shards them across cores by contiguous
    dst-node ranges (6272 local nodes = 49 blocks of 128 per core), so each
    core computes complete output rows for its dst range.
  - Phase A (per core): dense matmuls produce a bf16 K||V table for ALL
    nodes and a pre-scaled bf16 Q table for the local nodes, both in DRAM.
  - Phase B (per 128-node block): per-edge work in edge-major layout
    [128 edges/partition-lanes x features]. Edge features e = ea@We.T (+bias)
    are computed by PE into the gather destination, then an indirect DMA
    gather with compute_op=add accumulates K[src]||V[src] on top (kj,vj for
    free). alpha = sum(q[dst] * kj) via scalar_tensor_tensor accum; softmax
    without max-subtraction (alpha is bounded); segment sums via a one-hot
    selection-matrix matmul into PSUM; beta-gated skip + proj per block.
"""

import sys

sys.path.insert(0, "/opt/trn_rl_repo")

import numpy as np

N, E, D, H, ED = 50000, 600000, 128, 2, 5
C = D // H
NCORES = 8
P = 128
NB = 49                 # node blocks per core
L = NB * P              # 6272 local nodes per core
NPAD = 392 * P          # 50176 padded node count (multiple of 512 too)
QSCALE = 0.125          # 1/sqrt(C)


def _bf16(a):
    import ml_dtypes

    return np.asarray(a, dtype=np.float32).astype(ml_dtypes.bfloat16)


def _prep_host(x, edge_index, edge_attr, Wq, bq, Wk, bk, Wv, bv, We,
               Wskip, bskip, Wbeta, Wproj, bproj):
    """Sort/shard edges, build per-core device arrays + shared consts."""
    src = np.asarray(edge_index[0], dtype=np.int64)
    dst = np.asarray(edge_index[1], dtype=np.int64)
    ea = np.asarray(edge_attr, dtype=np.float32)

    core_of = dst // L
    blk_of = (dst % L) // P

    # per (core, block) edge lists
    order = np.lexsort((src, blk_of, core_of))
    s_src, s_dst, s_core, s_blk = src[order], dst[order], core_of[order], blk_of[order]
    s_ea = ea[order]

    counts_lo = np.zeros((NCORES, NB), dtype=np.int64)
    counts_hi = np.zeros((NCORES, NB), dtype=np.int64)
    lo_mask = s_src < 32768
    np.add.at(counts_lo, (s_core[lo_mask], s_blk[lo_mask]), 1)
    np.add.at(counts_hi, (s_core[~lo_mask], s_blk[~lo_mask]), 1)
    Tlo = -(-counts_lo.max(axis=0) // P)
    Thi = -(-counts_hi.max(axis=0) // P)
    Tlo = np.where((Tlo + Thi) == 0, 1, Tlo)     # at least one tile per block
    Tb = Tlo + Thi
    offs = np.concatenate([[0], np.cumsum(Tb)])
    offs_lo = np.concatenate([[0], np.cumsum(Tlo)])
    offs_hi = np.concatenate([[0], np.cumsum(Thi)])
    sumT, sumTl, sumTh = int(offs[-1]), int(offs_lo[-1]), int(offs_hi[-1])

    dstl = np.full((NCORES, P, sumT), 300.0, dtype=np.float32)
    eaT6 = np.zeros((NCORES, 6, sumT * P), dtype=np.float32)
    eaT6[:, 5, :] = 1.0
    kvia = np.zeros((NCORES, P, sumTl * 8), dtype=np.int16)
    kvib = np.zeros((NCORES, P, sumTh * 8), dtype=np.int16)
    qidx = np.zeros((NCORES, P, sumT * 8), dtype=np.int16)

    def wrap16(flat):
        # edge i -> [i%16, i//16], replicated over 8 partition groups
        w = flat.reshape(-1, 16).T.astype(np.int16)      # [16, n/16]
        return np.tile(w, (8, 1))

    # group edges by (core, block, low/high); they are sorted by src already
    c_all = s_core * NB + s_blk
    for c in range(NCORES):
        for b in range(NB):
            sel = (s_core == c) & (s_blk == b)
            esrc, edst, eea = s_src[sel], s_dst[sel], s_ea[sel]
            nlo = int((esrc < 32768).sum())
            T, Tl, Th = int(Tb[b]), int(Tlo[b]), int(Thi[b])
            fsrc = np.zeros(T * P, np.int64)
            fsrc[Tl * P:] = 32768
            fdq = np.zeros(T * P, np.int64)
            fdl = np.full(T * P, 300.0, np.float32)
            fea = np.zeros((T * P, 5), np.float32)
            fsrc[:nlo] = esrc[:nlo]
            fdq[:nlo] = edst[:nlo] - c * L
            fdl[:nlo] = (edst[:nlo] - c * L - b * P).astype(np.float32)
            fea[:nlo] = eea[:nlo]
            nhi = len(esrc) - nlo
            if nhi:
                hs = slice(Tl * P, Tl * P + nhi)
                fsrc[hs] = esrc[nlo:]
                fdq[hs] = edst[nlo:] - c * L
                fdl[hs] = (edst[nlo:] - c * L - b * P).astype(np.float32)
                fea[hs] = eea[nlo:]
            o = offs[b]
            dstl[c, :, o:o + T] = fdl.reshape(T, P).T
            eaT6[c, :5, o * P:(o + T) * P] = fea.T
            if Tl:
                kvia[c, :, offs_lo[b] * 8:(offs_lo[b] + Tl) * 8] = wrap16(fsrc[:Tl * P])
            if Th:
                kvib[c, :, offs_hi[b] * 8:(offs_hi[b] + Th) * 8] = \
                    wrap16(fsrc[Tl * P:] - 32768)
            qidx[c, :, o * 8:(o + T) * 8] = wrap16(fdq)

    xpad = np.zeros((NPAD, D), dtype=np.float32)
    xpad[:N] = np.asarray(x, dtype=np.float32)
    xT = _bf16(xpad.T)                                   # [128, NPAD]

    xTloc = np.zeros((NCORES, D, L), dtype=np.float32)
    for c in range(NCORES):
        hi = min(N, (c + 1) * L)
        if hi > c * L:
            xTloc[c, :, : hi - c * L] = xpad[c * L: hi].T
    xTloc = _bf16(xTloc)

    Wb = np.asarray(Wbeta, dtype=np.float32).reshape(3, D)
    consts = {
        "wkvt": _bf16(np.concatenate([np.asarray(Wk).T, np.asarray(Wv).T], axis=1)),
        "wqt": _bf16(np.asarray(Wq).T),
        "bqrow": _bf16(np.asarray(bq).reshape(1, D)),
        "we2": _bf16(np.concatenate(
            [np.concatenate([np.asarray(We).T, np.asarray(We).T], axis=1),
             np.concatenate([np.asarray(bk), np.asarray(bv)]).reshape(1, 2 * D)],
            axis=0)),                                    # [6, 256]
        "wskipt": _bf16(np.asarray(Wskip).T),
        "bskiprow": _bf16(np.asarray(bskip).reshape(1, D)),
        "wprojt": _bf16(np.asarray(Wproj).T),
        "bprojrow": _bf16(np.asarray(bproj).reshape(1, D)),
        "wb1rep": _bf16(np.tile((Wb[0] + Wb[2]).reshape(1, D), (P, 1))),
        "wb2rep": _bf16(np.tile((Wb[1] - Wb[2]).reshape(1, D), (P, 1))),
        "iota": _bf16(np.tile(np.arange(P, dtype=np.float32).reshape(1, P), (P, 1))),
        "onesrow": _bf16(np.ones((1, D), dtype=np.float32)),
    }

    per_core = []
    for c in range(NCORES):
        m = dict(consts)
        m["xt"] = xT
        m["xtloc"] = xTloc[c]
        m["kvia"] = kvia[c] if sumTl else np.zeros((P, 1), np.int16)
        m["kvib"] = kvib[c] if sumTh else np.zeros((P, 1), np.int16)
        m["qidx"] = qidx[c]
        m["dstl"] = _bf16(dstl[c])
        m["eat6"] = _bf16(eaT6[c])
        per_core.append(m)
    meta = dict(Tb=[int(t) for t in Tb], Tlo=[int(t) for t in Tlo],
                offs=[int(o) for o in offs],
                offs_lo=[int(o) for o in offs_lo],
                offs_hi=[int(o) for o in offs_hi])
    return per_core, meta


def _build_program(meta):
    import os
    STAGE = int(os.environ.get('BISECT_STAGE', '5'))
    DUMP = os.environ.get('DUMP_TENSOR', '')
    Tb, Tlo = meta["Tb"], meta["Tlo"]
    offs, offs_lo, offs_hi = meta["offs"], meta["offs_lo"], meta["offs_hi"]
    import concourse.bacc as bacc
    import concourse.bass as bass
    import concourse.mybir as mybir
    import concourse.tile as tile
    from concourse.masks import make_identity

    fp32 = mybir.dt.float32
    bf16 = mybir.dt.bfloat16
    i32 = mybir.dt.int32
    AX = mybir.AluOpType
    AF = mybir.ActivationFunctionType
    sumT = offs[-1]
    sumTl, sumTh = offs_lo[-1], offs_hi[-1]
    i16 = mybir.dt.int16

    nc = bacc.Bacc("TRN2", target_bir_lowering=False, num_devices=NCORES)

    # ---------- parameters ----------
    xt = nc.declare_dram_parameter("xt", [D, NPAD], bf16, isOutput=False)
    xtloc = nc.declare_dram_parameter("xtloc", [D, L], bf16, isOutput=False)
    kvia = nc.declare_dram_parameter("kvia", [P, max(1, sumTl * 8)], i16, isOutput=False)
    kvib = nc.declare_dram_parameter("kvib", [P, max(1, sumTh * 8)], i16, isOutput=False)
    qidx = nc.declare_dram_parameter("qidx", [P, sumT * 8], i16, isOutput=False)
    dstl = nc.declare_dram_parameter("dstl", [P, sumT], bf16, isOutput=False)
    eat6 = nc.declare_dram_parameter("eat6", [6, sumT * P], bf16, isOutput=False)
    wkvt = nc.declare_dram_parameter("wkvt", [D, 2 * D], bf16, isOutput=False)
    wqt = nc.declare_dram_parameter("wqt", [D, D], bf16, isOutput=False)
    bqrow = nc.declare_dram_parameter("bqrow", [1, D], bf16, isOutput=False)
    we2 = nc.declare_dram_parameter("we2", [6, 2 * D], bf16, isOutput=False)
    wskipt = nc.declare_dram_parameter("wskipt", [D, D], bf16, isOutput=False)
    bskiprow = nc.declare_dram_parameter("bskiprow", [1, D], bf16, isOutput=False)
    wprojt = nc.declare_dram_parameter("wprojt", [D, D], bf16, isOutput=False)
    bprojrow = nc.declare_dram_parameter("bprojrow", [1, D], bf16, isOutput=False)
    wb1rep = nc.declare_dram_parameter("wb1rep", [P, D], bf16, isOutput=False)
    wb2rep = nc.declare_dram_parameter("wb2rep", [P, D], bf16, isOutput=False)
    iota = nc.declare_dram_parameter("iota", [P, P], bf16, isOutput=False)
    onesrow = nc.declare_dram_parameter("onesrow", [1, D], bf16, isOutput=False)
    out = nc.declare_dram_parameter("out", [L, D], fp32, isOutput=True)

    LO = min(32768, NPAD)
    kvta = nc.dram_tensor("kvta", [LO, 2 * D], bf16)
    kvtb = nc.dram_tensor("kvtb", [max(512, NPAD - LO), 2 * D], bf16)
    qt = nc.dram_tensor("qt", [L, 2 * D], bf16)

    with tile.TileContext(nc) as tc:
        # ================= Phase A: node projections =================
        with tc.tile_pool(name="pa", bufs=3) as pa, \
             tc.tile_pool(name="pac", bufs=1) as pac, \
             tc.tile_pool(name="pap", bufs=2, space="PSUM") as pap:
            wkvt_sb = pac.tile([D, 2 * D], bf16)
            nc.sync.dma_start(out=wkvt_sb[:], in_=wkvt[:])
            wqt_sb = pac.tile([D, D], bf16)
            nc.sync.dma_start(out=wqt_sb[:], in_=wqt[:])
            bq_sb = pac.tile([1, D], bf16)
            nc.sync.dma_start(out=bq_sb[:], in_=bqrow[:])
            ones_sb = pac.tile([1, D], bf16)
            nc.sync.dma_start(out=ones_sb[:], in_=onesrow[:])

            G = NPAD // 512  # 98 groups of 4 node-tiles
            for g in range(G):
                xt_t = pa.tile([D, 512], bf16, tag="xt_t")
                nc.sync.dma_start(out=xt_t[:], in_=xt[:, g * 512:(g + 1) * 512])
                kv_ps = pap.tile([P, 1024], fp32, tag="kv_ps")
                for s in range(4):
                    nc.tensor.matmul(
                        out=kv_ps[:, s * 256:(s + 1) * 256],
                        lhsT=xt_t[:, s * 128:(s + 1) * 128],
                        rhs=wkvt_sb[:], start=True, stop=True)
                kv_sb = pa.tile([P, 1024], bf16, tag="kv_sb")
                if g % 2 == 0:
                    nc.scalar.copy(out=kv_sb[:], in_=kv_ps[:])
                else:
                    nc.vector.tensor_copy(kv_sb[:], kv_ps[:])
                if (g + 1) * 512 <= LO:
                    kv_dst = kvta[g * 512:(g + 1) * 512, :]
                else:
                    kv_dst = kvtb[g * 512 - LO:(g + 1) * 512 - LO, :]
                nc.sync.dma_start(
                    out=kv_dst.rearrange("(s n) d -> n s d", s=4),
                    in_=kv_sb[:].rearrange("n (s d) -> n s d", s=4))

            for t in range(NB):
                xq_t = pa.tile([D, P], bf16, tag="xq_t")
                nc.sync.dma_start(out=xq_t[:], in_=xtloc[:, t * P:(t + 1) * P])
                q_ps = pap.tile([P, D], fp32, tag="q_ps")
                nc.tensor.matmul(out=q_ps[:], lhsT=xq_t[:], rhs=wqt_sb[:],
                                 start=True, stop=False)
                nc.tensor.matmul(out=q_ps[:], lhsT=ones_sb[:], rhs=bq_sb[:],
                                 start=False, stop=True)
                q_sb = pa.tile([P, 2 * D], bf16, tag="q_sb")
                nc.scalar.activation(
                    q_sb[:].rearrange("p (s d) -> p s d", s=2),
                    q_ps[:, None, :].to_broadcast([P, 2, D]),
                    AF.Copy, scale=QSCALE)
                nc.sync.dma_start(out=qt[t * P:(t + 1) * P, :], in_=q_sb[:])

        tc.strict_bb_all_engine_barrier()

        # ================= Phase B: edge aggregation =================
        with tc.tile_pool(name="pbc", bufs=1) as pbc, \
             tc.tile_pool(name="pb", bufs=2) as pb, \
             tc.tile_pool(name="pbs", bufs=4) as pbs, \
             tc.tile_pool(name="pbp", bufs=2, space="PSUM") as pbp, \
             tc.tile_pool(name="pbe", bufs=2, space="PSUM") as pbe, \
             tc.tile_pool(name="pbq", bufs=1, space="PSUM") as pbq:
            we2_sb = pbc.tile([6, 2 * D], bf16)
            nc.sync.dma_start(out=we2_sb[:], in_=we2[:])
            iota_sb = pbc.tile([P, P], bf16)
            nc.sync.dma_start(out=iota_sb[:], in_=iota[:])
            wsk_sb = pbc.tile([D, D], bf16)
            nc.sync.dma_start(out=wsk_sb[:], in_=wskipt[:])
            bsk_sb = pbc.tile([1, D], bf16)
            nc.sync.dma_start(out=bsk_sb[:], in_=bskiprow[:])
            wpr_sb = pbc.tile([D, D], bf16)
            nc.sync.dma_start(out=wpr_sb[:], in_=wprojt[:])
            bpr_sb = pbc.tile([1, D], bf16)
            nc.sync.dma_start(out=bpr_sb[:], in_=bprojrow[:])
            wb1_sb = pbc.tile([P, D], bf16)
            nc.sync.dma_start(out=wb1_sb[:], in_=wb1rep[:])
            wb2_sb = pbc.tile([P, D], bf16)
            nc.sync.dma_start(out=wb2_sb[:], in_=wb2rep[:])
            ones2_sb = pbc.tile([1, D], bf16)
            nc.sync.dma_start(out=ones2_sb[:], in_=onesrow[:])
            ident_sb = pbc.tile([P, P], bf16)
            make_identity(nc, ident_sb[:])

            for b in range(NB):
                T = Tb[b]
                o = offs[b]
                dstl_t = pb.tile([P, T], bf16, tag="dstl_t")
                nc.sync.dma_start(out=dstl_t[:], in_=dstl[:, o:o + T])
                Tl = Tlo[b]
                Th = T - Tl
                ol, oh = offs_lo[b], offs_hi[b]
                if Tl:
                    kvia_t = pb.tile([P, Tl * 8], i16, tag="kvia_t")
                    nc.sync.dma_start(out=kvia_t[:],
                                      in_=kvia[:, ol * 8:(ol + Tl) * 8])
                if Th:
                    kvib_t = pb.tile([P, Th * 8], i16, tag="kvib_t")
                    nc.sync.dma_start(out=kvib_t[:],
                                      in_=kvib[:, oh * 8:(oh + Th) * 8])
                qidx_t = pb.tile([P, T * 8], i16, tag="qidx_t")
                nc.sync.dma_start(out=qidx_t[:], in_=qidx[:, o * 8:(o + T) * 8])
                ea_t = pb.tile([6, T * P], bf16, tag="ea_t")
                nc.sync.dma_start(out=ea_t[:], in_=eat6[:, o * P:(o + T) * P])

                if STAGE < 2:
                    continue
                kvj = pb.tile([P, T * 256], bf16, tag="kvj")
                kvg = pb.tile([P, T * 256], bf16, tag="kvg")
                e2s = pb.tile([P, T * 256], bf16, tag="e2s")
                alpha = pb.tile([P, 2 * T], fp32, tag="alpha")
                ex = pb.tile([P, 2 * T], bf16, tag="ex")
                s2 = pb.tile([P, T * P], bf16, tag="s2")
                xmat = pb.tile([P, T * 132], bf16, tag="xmat")

                # e2 = ea@We.T (+bk||bv) -> kvj, then gather-add K||V rows
                for t in range(T):
                    e2_ps = pbe.tile([P, 256], fp32, tag="e2")
                    nc.tensor.matmul(out=e2_ps[:],
                                     lhsT=ea_t[:, t * P:(t + 1) * P],
                                     rhs=we2_sb[:], start=True, stop=True)
                    if t % 2 == 0:
                        nc.scalar.copy(out=e2s[:, t * 256:(t + 1) * 256],
                                       in_=e2_ps[:])
                    else:
                        nc.vector.tensor_copy(e2s[:, t * 256:(t + 1) * 256],
                                              e2_ps[:])
                def chunked_gather(dst, dst_off_tiles, table, idx_sb, ntiles):
                    for c0 in range(0, ntiles, 8):
                        cn = min(8, ntiles - c0)
                        nc.gpsimd.dma_gather(
                            out_ap=dst[:, (dst_off_tiles + c0) * 256:
                                       (dst_off_tiles + c0 + cn) * 256].rearrange(
                                "p (t d) -> p t d", d=256),
                            in_ap=table[:],
                            idxs_ap=idx_sb[:, c0 * 8:(c0 + cn) * 8],
                            num_idxs=cn * P, num_idxs_reg=cn * P,
                            elem_size=256)

                if STAGE >= 3:
                    if Tl:
                        chunked_gather(kvg, 0, kvta, kvia_t, Tl)
                    if Th:
                        chunked_gather(kvg, Tl, kvtb, kvib_t, Th)
                    nc.vector.tensor_tensor(out=kvj[:], in0=kvg[:], in1=e2s[:],
                                            op=AX.add)
                qg2 = pb.tile([P, T * 256], bf16, tag="qg2")
                chunked_gather(qg2, 0, qt, qidx_t, T)

                # alpha[e,h] = sum_d q*kj  (q pre-scaled by 1/8)
                if STAGE < 4:
                    continue
                for t in range(T):
                    for h in range(H):
                        sc = pbs.tile([P, C], bf16, tag="sc")
                        nc.vector.scalar_tensor_tensor(
                            out=sc[:],
                            in0=qg2[:, t * 256 + h * C: t * 256 + (h + 1) * C],
                            scalar=1.0,
                            in1=kvj[:, t * 256 + h * C: t * 256 + (h + 1) * C],
                            op0=AX.bypass, op1=AX.mult,
                            accum_out=alpha[:, 2 * t + h: 2 * t + h + 1])
                nc.scalar.activation(ex[:], alpha[:], AF.Exp)

                if STAGE < 5:
                    continue
                # selection matrix S2[e, n] = (dstl[e] == n)
                nc.vector.tensor_tensor(
                    out=s2[:].rearrange("p (t n) -> p t n", t=T),
                    in0=dstl_t[:, :, None].to_broadcast([P, T, P]),
                    in1=iota_sb[:, None, :].to_broadcast([P, T, P]),
                    op=AX.is_equal)

                # X = [vj * ex_h || ex]
                xv = xmat[:].rearrange("p (t f) -> p t f", t=T)
                kv_v = kvj[:].rearrange("p (t f) -> p t f", t=T)
                exg = ex[:].rearrange("p (t h) -> p t h", t=T)
                for h in range(H):
                    nc.vector.tensor_tensor(
                        out=xv[:, :, h * C:(h + 1) * C],
                        in0=kv_v[:, :, D + h * C: D + (h + 1) * C],
                        in1=exg[:, :, h: h + 1].to_broadcast([P, T, C]),
                        op=AX.mult)
                nc.vector.tensor_copy(xv[:, :, D: D + 2], exg[:])

                # segment sums into PSUM via S2^T @ X
                acc_ps = pbp.tile([P, 130], fp32, tag="acc")
                for t in range(T):
                    nc.tensor.matmul(out=acc_ps[:],
                                     lhsT=s2[:, t * P:(t + 1) * P],
                                     rhs=xmat[:, t * 132: t * 132 + 130],
                                     start=(t == 0), stop=(t == T - 1))

                # normalize + beta-gated skip + proj
                den = pbs.tile([P, 2], fp32, tag="den")
                nc.vector.tensor_scalar_add(den[:], acc_ps[:, D:D + 2], 1e-30)
                denr = pbs.tile([P, 2], fp32, tag="denr")
                nc.vector.reciprocal(denr[:], den[:])
                oa = pbs.tile([P, D], bf16, tag="oa")
                for h in range(H):
                    nc.vector.tensor_scalar_mul(
                        oa[:, h * C:(h + 1) * C],
                        acc_ps[:, h * C:(h + 1) * C],
                        denr[:, h: h + 1])

                xr_t = pbs.tile([D, P], bf16, tag="xr_t")
                nc.sync.dma_start(out=xr_t[:], in_=xtloc[:, b * P:(b + 1) * P])
                xr_ps = pbq.tile([P, D], fp32, tag="xr_ps")
                nc.tensor.matmul(out=xr_ps[:], lhsT=xr_t[:], rhs=wsk_sb[:],
                                 start=True, stop=False)
                nc.tensor.matmul(out=xr_ps[:], lhsT=ones2_sb[:], rhs=bsk_sb[:],
                                 start=False, stop=True)
                xr_sb = pbs.tile([P, D], bf16, tag="xr_sb")
                nc.scalar.copy(out=xr_sb[:], in_=xr_ps[:])

                bp = pbs.tile([P, 2], fp32, tag="bp")
                sc2 = pbs.tile([P, D], bf16, tag="sc2")
                nc.vector.scalar_tensor_tensor(
                    out=sc2[:], in0=oa[:], scalar=1.0, in1=wb1_sb[:],
                    op0=AX.bypass, op1=AX.mult, accum_out=bp[:, 0:1])
                sc3 = pbs.tile([P, D], bf16, tag="sc3")
                nc.vector.scalar_tensor_tensor(
                    out=sc3[:], in0=xr_sb[:], scalar=1.0, in1=wb2_sb[:],
                    op0=AX.bypass, op1=AX.mult, accum_out=bp[:, 1:2])
                beta = pbs.tile([P, 1], fp32, tag="beta")
                nc.scalar.activation(beta[:], bp[:, 0:1], AF.Sigmoid,
                                     bias=bp[:, 1:2])

                diff = pbs.tile([P, D], bf16, tag="diff")
                nc.vector.tensor_tensor(out=diff[:], in0=xr_sb[:], in1=oa[:],
                                        op=AX.subtract)
                y_sb = pbs.tile([P, D], bf16, tag="y_sb")
                nc.vector.scalar_tensor_tensor(
                    out=y_sb[:], in0=diff[:], scalar=beta[:, 0:1], in1=oa[:],
                    op0=AX.mult, op1=AX.add)

                yt_ps = pbq.tile([P, D], bf16, tag="yt_ps")
                nc.tensor.transpose(out=yt_ps[:], in_=y_sb[:], identity=ident_sb[:])
                yt_sb = pbs.tile([P, D], bf16, tag="yt_sb")
                nc.scalar.copy(out=yt_sb[:], in_=yt_ps[:])
                yp_ps = pbq.tile([P, D], fp32, tag="yp_ps")
                nc.tensor.matmul(out=yp_ps[:], lhsT=yt_sb[:], rhs=wpr_sb[:],
                                 start=True, stop=False)
                nc.tensor.matmul(out=yp_ps[:], lhsT=ones2_sb[:], rhs=bpr_sb[:],
                                 start=False, stop=True)
                o_sb = pbs.tile([P, D], fp32, tag="o_sb")
                if b % 2 == 0:
                    nc.scalar.copy(out=o_sb[:], in_=yp_ps[:])
                else:
                    nc.vector.tensor_copy(o_sb[:], yp_ps[:])
                if DUMP and b == 0:
                    dmp = pbs.tile([P, D], fp32, tag="dmp")
                    nc.gpsimd.memset(dmp[:], 0)
                    srcs = {"kvg": kvg[:, :D], "e2s": e2s[:, :D],
                            "kvj": kvj[:, :D], "qg": qg2[:, :D],
                            "alpha": alpha[:], "ex": ex[:],
                            "s2": s2[:, :D], "xmat": xmat[:, :D]}
                    sap = srcs[DUMP]
                    nc.vector.tensor_copy(dmp[:, :sap.shape[-1]], sap)
                    nc.sync.dma_start(out=out[b * P:(b + 1) * P, :], in_=dmp[:])
                else:
                    nc.sync.dma_start(out=out[b * P:(b + 1) * P, :], in_=o_sb[:])

    nc.compile()
    return nc


_CACHE = {}


def kernel(**inputs):
    from concourse.bass_utils import run_bass_kernel_spmd

    per_core, meta = _prep_host(**inputs)
    key = (tuple(meta["Tb"]), tuple(meta["Tlo"]))
    if key not in _CACHE:
        _CACHE[key] = _build_program(meta)
    nc = _CACHE[key]
    res = run_bass_kernel_spmd(nc, per_core, core_ids=list(range(NCORES)))
    full = np.concatenate([res.results[c]["out"] for c in range(NCORES)], axis=0)
    return np.ascontiguousarray(full[:N]).astype(np.float32)
